# revision 14
# baseline (speedup 1.0000x reference)
"""Trainium2 Bass kernel for nn_LogicConv3d (differentiable logic-gate 3D conv).

Architecture (v3)
-----------------
Each tree node out = c0 + ca*a + cb*b + cab*a*b is evaluated as
    u     = CAB*wX + CX2        (affine of one child)
    w_out = alpha * u           (tensor_tensor, 2x perf mode)
    alpha = s*wY + r            (affine of the other child)
with the per-node constant this factorization introduces (delta=CX2*CY2/CAB)
and the bilinear constant folded into the parent's coefficients host-side in
fp64 (fold2).  Per-node orientation (which child is X) minimizes |q|=|CY2/CAB|
over the 8 cores sharing the SPMD program; per-node scaling lam keeps
intermediates O(1) in fp16 (end-to-end rel err ~3e-3, tolerance 2e-2).

Level 0 reads pre-gathered window streams, and BOTH the u-affine and the
alpha-affine of every leaf are applied on the host while packing the streams:
level 0 on device is DMA -> one 8-wide TENSOR_TENSOR per oct.  Upper levels
need 2 affine (tensor_scalar / ACTIVATE) ops + a packed TT per node-group.

Engines: measured rates on (128,844) fp16 ops: DVE TS 494 / TT(oct) 3536 /
TT(quad) 1930 / TT(pair) 1040 / TT(single) 592; ACT 1078.  GPSIMD is NOT
used: its SBUF traffic degrades concurrent DVE ops 1.5-2.5x (measured), a
net loss.  Upper-level affines are greedily balanced DVE vs ACT.

Sharding: kernels K=32 split 4-per-core across 8 cores; positions packed as
(128 partitions x 844) fp16 tiles.  Per-core DMA: 27.6 MB of fp16 streams
(13KB-per-partition descriptors, auto-spread over 16 DMA queues).  Output:
one (128,844) fp16 tile per kernel; host applies v = w/lam + gam.
"""
import numpy as np

# ---- problem constants (hardcoded per contest contract) ----
B, C, H, W, D = 4, 3, 32, 32, 32
K, S = 32, 16
OH = OW = OD = 30
P = OH * OW * OD            # 27000
BP = B * P                  # 108000
NPART = 128
FREE = (BP + NPART - 1) // NPART   # 844
PADBP = NPART * FREE        # 108032
NCORES = 8
KLOC = K // NCORES          # 4
TEMP = 1.0
NLEV = 5
LEV_N = [16, 8, 4, 2, 1]    # nodes per kernel per level
TT_W = {1: 8, 2: 4, 3: 2, 4: 1}   # TT pack width per upper level

GATES = np.array([[(g >> t) & 1 for t in range(4)] for g in range(16)],
                 dtype=np.float64)

# measured per-op ns on (128,844) fp16 (solo DVE/ACT concurrency)
RATE_DVE_TS = 494.0
RATE_ACT_TS = 1078.0
RATE_TT = {8: 3536.0, 4: 1930.0, 2: 1040.0, 1: 592.0}


# ----------------------------------------------------------------- host math
def _lut_coeffs(w):
    w = w.astype(np.float64)
    e = np.exp((w - w.max(-1, keepdims=True)) / TEMP)
    p = e / e.sum(-1, keepdims=True)
    l = p @ GATES
    l0, l1, l2, l3 = l[..., 0], l[..., 1], l[..., 2], l[..., 3]
    return l0, l2 - l0, l1 - l0, l0 - l1 - l2 + l3


def fold2(ws):
    """Fold the tree for the 2-op node form.  Returns per-level dicts."""
    out = []
    for lev, w in enumerate(ws):
        c0, ca, cb, cab = _lut_coeffs(w)          # (nodes, K)
        n = c0.shape[0]
        if lev == 0:
            lamA = np.ones((n, K)); gamA = np.zeros((n, K))
            lamB = np.ones((n, K)); gamB = np.zeros((n, K))
            wloA = np.zeros((n, K)); whiA = np.ones((n, K))
            wloB = np.zeros((n, K)); whiB = np.ones((n, K))
        else:
            lam_p, gam_p = out[-1]["lam"], out[-1]["gam"]
            wlo_p, whi_p = out[-1]["wlo"], out[-1]["whi"]
            lamA, lamB = lam_p[0::2], lam_p[1::2]
            gamA, gamB = gam_p[0::2], gam_p[1::2]
            wloA, whiA = wlo_p[0::2], whi_p[0::2]
            wloB, whiB = wlo_p[1::2], whi_p[1::2]

        CAB = cab / (lamA * lamB)
        CA = (ca + cab * gamB) / lamA
        CB = (cb + cab * gamA) / lamB
        C0p = c0 + ca * gamA + cb * gamB + cab * gamA * gamB
        delta = CA * CB / CAB

        qXA = CA / CAB   # q if X=A child (shift B)
        qXB = CB / CAB   # q if X=B child (shift A)
        swap = (np.abs(qXB).max(axis=1) < np.abs(qXA).max(axis=1))  # (nodes,)

        q = np.where(swap[:, None], qXB, qXA)
        wloY = np.where(swap[:, None], wloA, wloB)
        whiY = np.where(swap[:, None], whiA, whiB)
        CX2 = np.where(swap[:, None], CA, CB)
        alo, ahi = wloY + q, whiY + q
        amax = np.maximum(np.abs(alo), np.abs(ahi))
        s = 1.0 / np.maximum(amax, 1e-6)
        r = s * q
        lam = s
        gam = C0p - delta
        wlo = np.minimum(s * (0 - gam), s * (1 - gam))
        whi = np.maximum(s * (0 - gam), s * (1 - gam))
        out.append(dict(swap=swap, CAB=CAB, CX2=CX2, s=s, r=r,
                        lam=lam, gam=gam, wlo=wlo, whi=whi))
    return out


def _coef_cols(F, core):
    """Per-core coefficient column vector, in program emission order
    (upper levels only; level-0 affines are host-applied)."""
    cols = []
    for kk in range(KLOC):
        k = core * KLOC + kk
        for lev in range(1, NLEV):
            f = F[lev]
            for i in range(LEV_N[lev]):
                cols += [f["CAB"][i, k], f["CX2"][i, k],
                         f["s"][i, k], f["r"][i, k]]
    return np.asarray(cols, dtype=np.float32)


def _prep_inputs(x, kc, ws):
    """Build per-core in_maps + fold data.  Streams (per core, per kernel):
    u_in[2*kk+j]  = host-affined X windows  (CAB*win + CX2) for oct j
    y_in[2*kk+j]  = host-affined Y windows  (s*(win + q))   for oct j
    """
    F = fold2(ws)

    X81 = np.empty((3, 3, 3, 3, B, OH, OW, OD), np.float32)
    for c in range(3):
        for dh in range(3):
            for dw in range(3):
                for dd in range(3):
                    X81[c, dh, dw, dd] = x[:, c, dh:dh + 30, dw:dw + 30,
                                           dd:dd + 30]
    X81 = X81.reshape(81, BP).astype(np.float64)

    h_, w_, d_, c_ = kc[..., 0], kc[..., 1], kc[..., 2], kc[..., 3]
    sl = ((c_ * 3 + h_) * 3 + w_) * 3 + d_          # (2,K,S)

    f0 = F[0]
    swap0 = f0["swap"]                               # (16,)
    in_maps = []
    for core in range(NCORES):
        us = np.zeros((KLOC * 2, NPART, 8 * FREE), np.float16)
        ys = np.zeros((KLOC * 2, NPART, 8 * FREE), np.float16)
        for kk in range(KLOC):
            k = core * KLOC + kk
            for n in range(16):
                iX, iY = (1, 0) if swap0[n] else (0, 1)
                uvals = (X81[sl[iX, k, n]] * f0["CAB"][n, k]
                         + f0["CX2"][n, k]).astype(np.float16)
                avals = (X81[sl[iY, k, n]] * f0["s"][n, k]
                         + f0["r"][n, k]).astype(np.float16)
                j, m = divmod(n, 8)
                pad = np.zeros(PADBP, np.float16)
                pad[:BP] = uvals
                us[2 * kk + j, :, m * FREE:(m + 1) * FREE] = \
                    pad.reshape(NPART, FREE)
                pad = np.zeros(PADBP, np.float16)
                pad[:BP] = avals
                ys[2 * kk + j, :, m * FREE:(m + 1) * FREE] = \
                    pad.reshape(NPART, FREE)
        coefv = _coef_cols(F, core)
        coef = np.broadcast_to(coefv, (NPART, coefv.size)).copy()
        in_maps.append({"u_in": us, "y_in": ys, "coef": coef})
    return in_maps, F


# ------------------------------------------------------------ device program
def _build_program(swaps_upper, n_coef):
    """swaps_upper: {lev: tuple of bools} for levels 1..4."""
    import concourse.bacc as bacc
    import concourse.mybir as mybir
    from concourse.tile import TileContext

    f16 = mybir.dt.float16
    f32 = mybir.dt.float32
    Alu = mybir.AluOpType
    Act = mybir.ActivationFunctionType

    nc = bacc.Bacc()
    u_in = nc.declare_dram_parameter("u_in", [KLOC * 2, NPART, 8 * FREE], f16,
                                     isOutput=False)
    y_in = nc.declare_dram_parameter("y_in", [KLOC * 2, NPART, 8 * FREE], f16,
                                     isOutput=False)
    coef = nc.declare_dram_parameter("coef", [NPART, n_coef], f32,
                                     isOutput=False)
    out = nc.declare_dram_parameter("out", [KLOC, NPART, FREE], f16,
                                    isOutput=True)

    acc = {"dve": 0.0, "act": 0.0}

    with TileContext(nc) as tc:
        with (
            tc.tile_pool(name="cpool", bufs=1) as cpool,
            tc.tile_pool(name="spool", bufs=3) as spool,
            tc.tile_pool(name="wpool", bufs=1) as wpool,
            tc.tile_pool(name="opool", bufs=2) as opool,
        ):
            coef_sb = cpool.tile([NPART, n_coef], f32)
            nc.sync.dma_start(out=coef_sb[:], in_=coef[:])
            col = [0]

            def nxtcol():
                c = coef_sb[:, col[0]:col[0] + 1]
                col[0] += 1
                return c

            def ts_op(dst_ap, src_ap, s1, s2):
                if acc["dve"] + RATE_DVE_TS <= acc["act"] + RATE_ACT_TS:
                    acc["dve"] += RATE_DVE_TS
                    nc.vector.tensor_scalar(dst_ap, src_ap, s1, s2,
                                            Alu.mult, Alu.add)
                else:
                    acc["act"] += RATE_ACT_TS
                    nc.scalar.activation(dst_ap, src_ap, Act.Identity,
                                         bias=s2, scale=s1)

            # coefficient columns are laid out k-major; index directly
            def colap(idx):
                return coef_sb[:, idx:idx + 1]

            COLS_PER_K = 4 * sum(LEV_N[1:])

            def child(prev, width, i):
                t = prev[i // width]
                m = i % width
                return t[:, m * FREE:(m + 1) * FREE]

            state = {}   # kk -> (prev tiles, prev width)

            def stage(kk, lev):
                if lev == 0:
                    # split first kernels' level-0 into finer DMA+TT chunks so
                    # the pipeline starts after ~0.4MB instead of ~3.5MB
                    split = {0: 4, 1: 2}.get(kk, 1)
                    o0 = []
                    for j in range(2):
                        ut = spool.tile([NPART, 8 * FREE], f16, tag="us",
                                        name=f"u0_{kk}_{j}", bufs=3)
                        yt = spool.tile([NPART, 8 * FREE], f16, tag="ys",
                                        name=f"y0_{kk}_{j}", bufs=3)
                        o = wpool.tile([NPART, 8 * FREE], f16, tag=f"o0{j}",
                                       name=f"o0{j}_{kk}", bufs=2)
                        cw = 8 // split
                        for s in range(split):
                            sl_ = slice(s * cw * FREE, (s + 1) * cw * FREE)
                            nc.sync.dma_start(out=ut[:, sl_],
                                              in_=u_in[2 * kk + j][:, sl_])
                            nc.sync.dma_start(out=yt[:, sl_],
                                              in_=y_in[2 * kk + j][:, sl_])
                            nc.vector.tensor_tensor(o[:, sl_], yt[:, sl_],
                                                    ut[:, sl_], Alu.mult)
                            acc["dve"] += RATE_TT[cw]
                        o0.append(o)
                    state[kk] = (o0, 8)
                    return
                prev, pwidth = state[kk]
                nn = LEV_N[lev]
                wdt = min(TT_W[lev], nn)
                ntile = (nn + wdt - 1) // wdt
                col0 = kk * COLS_PER_K + 4 * sum(LEV_N[1:lev])
                ut = [wpool.tile([NPART, wdt * FREE], f16,
                                 tag=f"u{lev}{t}", name=f"u{lev}{t}_{kk}")
                      for t in range(ntile)]
                at = [wpool.tile([NPART, wdt * FREE], f16,
                                 tag=f"a{lev}{t}", name=f"a{lev}{t}_{kk}")
                      for t in range(ntile)]
                ot = []
                sw = swaps_upper[lev]
                for t in range(ntile):
                    for m in range(wdt):
                        i = t * wdt + m
                        s1, s2, s3, s4 = (colap(col0 + 4 * i),
                                          colap(col0 + 4 * i + 1),
                                          colap(col0 + 4 * i + 2),
                                          colap(col0 + 4 * i + 3))
                        iX = 2 * i + (1 if sw[i] else 0)
                        iY = 2 * i + (0 if sw[i] else 1)
                        ts_op(ut[t][:, m * FREE:(m + 1) * FREE],
                              child(prev, pwidth, iX), s1, s2)
                        ts_op(at[t][:, m * FREE:(m + 1) * FREE],
                              child(prev, pwidth, iY), s3, s4)
                    if lev == NLEV - 1:
                        ot_k = opool.tile([NPART, FREE], f16, tag="ot",
                                          name=f"ot{kk}", bufs=1)
                        nc.vector.tensor_tensor(ot_k[:], at[t][:],
                                                ut[t][:], Alu.mult)
                        acc["dve"] += RATE_TT[wdt]
                        nc.sync.dma_start(out=out[kk], in_=ot_k[:])
                    else:
                        o = wpool.tile([NPART, wdt * FREE], f16,
                                       tag=f"o{lev}{t}",
                                       name=f"o{lev}{t}_{kk}")
                        nc.vector.tensor_tensor(o[:], at[t][:], ut[t][:],
                                                Alu.mult)
                        acc["dve"] += RATE_TT[wdt]
                        ot.append(o)
                state[kk] = (ot, wdt)

            # software-pipelined (kernel, level) wavefront: keeps each
            # engine's in-order queue stocked with independent work
            ORDER = [(0, 0), (1, 0), (0, 1), (0, 2), (1, 1), (2, 0),
                     (0, 3), (1, 2), (0, 4), (2, 1), (3, 0), (1, 3),
                     (2, 2), (1, 4), (3, 1), (2, 3), (3, 2), (2, 4),
                     (3, 3), (3, 4)]
            for kk, lev in ORDER:
                stage(kk, lev)
    nc.compile()
    return nc


_PROGRAM = None
_PROGRAM_KEY = None


def _get_program(F, n_coef):
    global _PROGRAM, _PROGRAM_KEY
    swaps_upper = {lev: tuple(bool(v) for v in F[lev]["swap"])
                   for lev in range(1, NLEV)}
    key = (tuple(sorted(swaps_upper.items())), n_coef)
    if _PROGRAM is None or _PROGRAM_KEY != key:
        _PROGRAM = _build_program(swaps_upper, n_coef)
        _PROGRAM_KEY = key
    return _PROGRAM


def _postprocess(results, F):
    full = np.empty((K, PADBP), np.float32)
    lam = F[NLEV - 1]["lam"][0]      # (K,)
    gam = F[NLEV - 1]["gam"][0]
    for core in range(NCORES):
        o = np.asarray(results[core]["out"], dtype=np.float32)
        for kk in range(KLOC):
            k = core * KLOC + kk
            w = o[kk].reshape(PADBP)
            full[k] = w / np.float32(lam[k]) + np.float32(gam[k])
    out = full[:, :BP].reshape(K, B, OH, OW, OD).transpose(1, 0, 2, 3, 4)
    return np.ascontiguousarray(out)


def kernel(**inputs):
    x = np.asarray(inputs["x"], dtype=np.float32)
    kc = np.asarray(inputs["kernel_coords"])
    ws = [np.asarray(inputs[f"w{i}"]) for i in range(5)]

    in_maps, F = _prep_inputs(x, kc, ws)
    n_coef = in_maps[0]["coef"].shape[1]
    prog = _get_program(F, n_coef)

    from concourse.bass_utils import run_bass_kernel_spmd
    res = run_bass_kernel_spmd(prog, in_maps, list(range(NCORES)))
    return _postprocess(res.results, F)


# revision 19
# speedup vs baseline: 1.0276x; 1.0276x over previous
"""Trainium2 Bass kernel for nn_LogicConv3d (differentiable logic-gate 3D conv).

Architecture (v3)
-----------------
Each tree node out = c0 + ca*a + cb*b + cab*a*b is evaluated as
    u     = CAB*wX + CX2        (affine of one child)
    w_out = alpha * u           (tensor_tensor, 2x perf mode)
    alpha = s*wY + r            (affine of the other child)
with the per-node constant this factorization introduces (delta=CX2*CY2/CAB)
and the bilinear constant folded into the parent's coefficients host-side in
fp64 (fold2).  Per-node orientation (which child is X) minimizes |q|=|CY2/CAB|
over the 8 cores sharing the SPMD program; per-node scaling lam keeps
intermediates O(1) in fp16 (end-to-end rel err ~3e-3, tolerance 2e-2).

Level 0 reads pre-gathered window streams, and BOTH the u-affine and the
alpha-affine of every leaf are applied on the host while packing the streams:
level 0 on device is DMA -> one 8-wide TENSOR_TENSOR per oct.  Upper levels
need 2 affine (tensor_scalar / ACTIVATE) ops + a packed TT per node-group.

Engines: measured rates on (128,844) fp16 ops: DVE TS 494 / TT(oct) 3536 /
TT(quad) 1930 / TT(pair) 1040 / TT(single) 592; ACT 1078.  GPSIMD is NOT
used: its SBUF traffic degrades concurrent DVE ops 1.5-2.5x (measured), a
net loss.  Upper-level affines are greedily balanced DVE vs ACT.

Sharding: kernels K=32 split 4-per-core across 8 cores; positions packed as
(128 partitions x 844) fp16 tiles.  Per-core DMA: 27.6 MB of fp16 streams
(13KB-per-partition descriptors, auto-spread over 16 DMA queues).  Output:
one (128,844) fp16 tile per kernel; host applies v = w/lam + gam.
"""
import numpy as np

# ---- problem constants (hardcoded per contest contract) ----
B, C, H, W, D = 4, 3, 32, 32, 32
K, S = 32, 16
OH = OW = OD = 30
P = OH * OW * OD            # 27000
BP = B * P                  # 108000
NPART = 128
FREE = (BP + NPART - 1) // NPART   # 844
PADBP = NPART * FREE        # 108032
NCORES = 8
KLOC = K // NCORES          # 4
TEMP = 1.0
NLEV = 5
LEV_N = [16, 8, 4, 2, 1]    # nodes per kernel per level
TT_W = {1: 8, 2: 4, 3: 2, 4: 1}   # TT pack width per upper level

GATES = np.array([[(g >> t) & 1 for t in range(4)] for g in range(16)],
                 dtype=np.float64)

# measured per-op ns on (128,844) fp16 (solo DVE/ACT concurrency)
RATE_DVE_TS = 494.0     # mult+add (alpha affine)
RATE_ACT_TS = 1078.0
RATE_DVE_TS1 = 435.0    # add-only (u affine)
RATE_ACT_TS1 = 982.0
RATE_TT = {8: 3536.0, 4: 1930.0, 2: 1040.0, 1: 592.0}


# ----------------------------------------------------------------- host math
def _lut_coeffs(w):
    w = w.astype(np.float64)
    e = np.exp((w - w.max(-1, keepdims=True)) / TEMP)
    p = e / e.sum(-1, keepdims=True)
    l = p @ GATES
    l0, l1, l2, l3 = l[..., 0], l[..., 1], l[..., 2], l[..., 3]
    return l0, l2 - l0, l1 - l0, l0 - l1 - l2 + l3


def fold2(ws):
    """Fold the tree for the 2-op node form.  Returns per-level dicts."""
    out = []
    for lev, w in enumerate(ws):
        c0, ca, cb, cab = _lut_coeffs(w)          # (nodes, K)
        n = c0.shape[0]
        if lev == 0:
            lamA = np.ones((n, K)); gamA = np.zeros((n, K))
            lamB = np.ones((n, K)); gamB = np.zeros((n, K))
            wloA = np.zeros((n, K)); whiA = np.ones((n, K))
            wloB = np.zeros((n, K)); whiB = np.ones((n, K))
        else:
            lam_p, gam_p = out[-1]["lam"], out[-1]["gam"]
            wlo_p, whi_p = out[-1]["wlo"], out[-1]["whi"]
            lamA, lamB = lam_p[0::2], lam_p[1::2]
            gamA, gamB = gam_p[0::2], gam_p[1::2]
            wloA, whiA = wlo_p[0::2], whi_p[0::2]
            wloB, whiB = wlo_p[1::2], whi_p[1::2]

        CAB = cab / (lamA * lamB)
        CA = (ca + cab * gamB) / lamA
        CB = (cb + cab * gamA) / lamB
        C0p = c0 + ca * gamA + cb * gamB + cab * gamA * gamB
        delta = CA * CB / CAB

        qXA = CA / CAB   # q if X=A child (shift B)
        qXB = CB / CAB   # q if X=B child (shift A)
        swap = (np.abs(qXB).max(axis=1) < np.abs(qXA).max(axis=1))  # (nodes,)

        q = np.where(swap[:, None], qXB, qXA)
        wloY = np.where(swap[:, None], wloA, wloB)
        whiY = np.where(swap[:, None], whiA, whiB)
        CX2 = np.where(swap[:, None], CA, CB)
        alo, ahi = wloY + q, whiY + q
        amax = np.maximum(np.abs(alo), np.abs(ahi))
        s = 1.0 / np.maximum(amax, 1e-6)
        r = s * q
        lam = s
        gam = C0p - delta
        wlo = np.minimum(s * (0 - gam), s * (1 - gam))
        whi = np.maximum(s * (0 - gam), s * (1 - gam))
        out.append(dict(swap=swap, CAB=CAB, CX2=CX2, s=s, r=r,
                        lam=lam, gam=gam, wlo=wlo, whi=whi))

    # Top-down pass: fold each node's u-scale CAB into its X-child's
    # alpha-affine (the child's whole output scales by CAB; every node has
    # exactly one consumer).  Device u-affines become add-only.
    for lev in range(NLEV - 1, 0, -1):
        f = out[lev]
        fc = out[lev - 1]
        for i in range(LEV_N[lev]):
            iX = 2 * i + (1 if f["swap"][i] else 0)
            D = f["CAB"][i].copy()
            fc["s"][iX] *= D
            fc["r"][iX] *= D
            f["CAB"][i] = np.ones_like(D)
    return out


def _coef_cols(F, core):
    """Per-core coefficient column vector, in program emission order
    (upper levels only; level-0 affines are host-applied)."""
    cols = []
    for kk in range(KLOC):
        k = core * KLOC + kk
        for lev in range(1, NLEV):
            f = F[lev]
            for i in range(LEV_N[lev]):
                cols += [f["CAB"][i, k], f["CX2"][i, k],
                         f["s"][i, k], f["r"][i, k]]
    return np.asarray(cols, dtype=np.float32)


def _prep_inputs(x, kc, ws):
    """Build per-core in_maps + fold data.  Streams (per core, per kernel):
    u_in[2*kk+j]  = host-affined X windows  (CAB*win + CX2) for oct j
    y_in[2*kk+j]  = host-affined Y windows  (s*(win + q))   for oct j
    """
    F = fold2(ws)

    X81 = np.empty((3, 3, 3, 3, B, OH, OW, OD), np.float32)
    for c in range(3):
        for dh in range(3):
            for dw in range(3):
                for dd in range(3):
                    X81[c, dh, dw, dd] = x[:, c, dh:dh + 30, dw:dw + 30,
                                           dd:dd + 30]
    X81 = X81.reshape(81, BP).astype(np.float64)

    h_, w_, d_, c_ = kc[..., 0], kc[..., 1], kc[..., 2], kc[..., 3]
    sl = ((c_ * 3 + h_) * 3 + w_) * 3 + d_          # (2,K,S)

    f0 = F[0]
    swap0 = f0["swap"]                               # (16,)
    in_maps = []
    for core in range(NCORES):
        us = np.zeros((KLOC * 2, NPART, 8 * FREE), np.float16)
        ys = np.zeros((KLOC * 2, NPART, 8 * FREE), np.float16)
        for kk in range(KLOC):
            k = core * KLOC + kk
            for n in range(16):
                iX, iY = (1, 0) if swap0[n] else (0, 1)
                uvals = (X81[sl[iX, k, n]] * f0["CAB"][n, k]
                         + f0["CX2"][n, k]).astype(np.float16)
                avals = (X81[sl[iY, k, n]] * f0["s"][n, k]
                         + f0["r"][n, k]).astype(np.float16)
                j, m = divmod(n, 8)
                pad = np.zeros(PADBP, np.float16)
                pad[:BP] = uvals
                us[2 * kk + j, :, m * FREE:(m + 1) * FREE] = \
                    pad.reshape(NPART, FREE)
                pad = np.zeros(PADBP, np.float16)
                pad[:BP] = avals
                ys[2 * kk + j, :, m * FREE:(m + 1) * FREE] = \
                    pad.reshape(NPART, FREE)
        coefv = _coef_cols(F, core)
        coef = np.broadcast_to(coefv, (NPART, coefv.size)).copy()
        in_maps.append({"u_in": us, "y_in": ys, "coef": coef})
    return in_maps, F


# ------------------------------------------------------------ device program
def _build_program(swaps_upper, n_coef):
    """swaps_upper: {lev: tuple of bools} for levels 1..4."""
    import concourse.bacc as bacc
    import concourse.mybir as mybir
    from concourse.tile import TileContext

    f16 = mybir.dt.float16
    f32 = mybir.dt.float32
    Alu = mybir.AluOpType
    Act = mybir.ActivationFunctionType

    nc = bacc.Bacc()
    u_in = nc.declare_dram_parameter("u_in", [KLOC * 2, NPART, 8 * FREE], f16,
                                     isOutput=False)
    y_in = nc.declare_dram_parameter("y_in", [KLOC * 2, NPART, 8 * FREE], f16,
                                     isOutput=False)
    coef = nc.declare_dram_parameter("coef", [NPART, n_coef], f32,
                                     isOutput=False)
    out = nc.declare_dram_parameter("out", [KLOC, NPART, FREE], f16,
                                    isOutput=True)

    acc = {"dve": 0.0, "act": 0.0}

    with TileContext(nc) as tc:
        with (
            tc.tile_pool(name="cpool", bufs=1) as cpool,
            tc.tile_pool(name="spool", bufs=3) as spool,
            tc.tile_pool(name="wpool", bufs=1) as wpool,
            tc.tile_pool(name="opool", bufs=2) as opool,
        ):
            coef_sb = cpool.tile([NPART, n_coef], f32)
            nc.sync.dma_start(out=coef_sb[:], in_=coef[:])
            col = [0]

            def nxtcol():
                c = coef_sb[:, col[0]:col[0] + 1]
                col[0] += 1
                return c

            def ts_op(dst_ap, src_ap, s1, s2, force=None):
                """s1 None => add-only (u affine, scale pre-folded)."""
                add_only = s1 is None
                cd = RATE_DVE_TS1 if add_only else RATE_DVE_TS
                ca_ = RATE_ACT_TS1 if add_only else RATE_ACT_TS
                use_dve = (force == "dve" or
                           (force is None and
                            acc["dve"] + cd <= acc["act"] + ca_))
                if use_dve:
                    acc["dve"] += cd
                    if add_only:
                        nc.vector.tensor_scalar(dst_ap, src_ap, s2, None,
                                                Alu.add)
                    else:
                        nc.vector.tensor_scalar(dst_ap, src_ap, s1, s2,
                                                Alu.mult, Alu.add)
                else:
                    acc["act"] += ca_
                    nc.scalar.activation(dst_ap, src_ap, Act.Identity,
                                         bias=s2,
                                         scale=1.0 if add_only else s1)

            # coefficient columns are laid out k-major; index directly
            def colap(idx):
                return coef_sb[:, idx:idx + 1]

            COLS_PER_K = 4 * sum(LEV_N[1:])

            def child(prev, width, i):
                t = prev[i // width]
                m = i % width
                return t[:, m * FREE:(m + 1) * FREE]

            state = {}   # kk -> (prev tiles, prev width)

            def stage(kk, lev):
                if lev == 0:
                    # split first kernels' level-0 into finer DMA+TT chunks so
                    # the pipeline starts after ~0.4MB instead of ~3.5MB
                    split = {0: 4, 1: 2}.get(kk, 1)
                    o0 = []
                    for j in range(2):
                        ut = spool.tile([NPART, 8 * FREE], f16, tag="us",
                                        name=f"u0_{kk}_{j}", bufs=3)
                        yt = spool.tile([NPART, 8 * FREE], f16, tag="ys",
                                        name=f"y0_{kk}_{j}", bufs=3)
                        o = wpool.tile([NPART, 8 * FREE], f16, tag=f"o0{j}",
                                       name=f"o0{j}_{kk}", bufs=2)
                        cw = 8 // split
                        for s in range(split):
                            sl_ = slice(s * cw * FREE, (s + 1) * cw * FREE)
                            nc.sync.dma_start(out=ut[:, sl_],
                                              in_=u_in[2 * kk + j][:, sl_])
                            nc.sync.dma_start(out=yt[:, sl_],
                                              in_=y_in[2 * kk + j][:, sl_])
                            nc.vector.tensor_tensor(o[:, sl_], yt[:, sl_],
                                                    ut[:, sl_], Alu.mult)
                            acc["dve"] += RATE_TT[cw]
                        o0.append(o)
                    state[kk] = (o0, 8)
                    return
                prev, pwidth = state[kk]
                nn = LEV_N[lev]
                wdt = min(TT_W[lev], nn)
                ntile = (nn + wdt - 1) // wdt
                col0 = kk * COLS_PER_K + 4 * sum(LEV_N[1:lev])
                ut = [wpool.tile([NPART, wdt * FREE], f16,
                                 tag=f"u{lev}{t}", name=f"u{lev}{t}_{kk}")
                      for t in range(ntile)]
                at = [wpool.tile([NPART, wdt * FREE], f16,
                                 tag=f"a{lev}{t}", name=f"a{lev}{t}_{kk}")
                      for t in range(ntile)]
                ot = []
                sw = swaps_upper[lev]
                force = "dve" if (kk == KLOC - 1 and lev >= 3) else None
                for t in range(ntile):
                    for m in range(wdt):
                        i = t * wdt + m
                        s2, s3, s4 = (colap(col0 + 4 * i + 1),
                                      colap(col0 + 4 * i + 2),
                                      colap(col0 + 4 * i + 3))
                        iX = 2 * i + (1 if sw[i] else 0)
                        iY = 2 * i + (0 if sw[i] else 1)
                        ts_op(ut[t][:, m * FREE:(m + 1) * FREE],
                              child(prev, pwidth, iX), None, s2, force)
                        ts_op(at[t][:, m * FREE:(m + 1) * FREE],
                              child(prev, pwidth, iY), s3, s4, force)
                    if lev == NLEV - 1:
                        ot_k = opool.tile([NPART, FREE], f16, tag="ot",
                                          name=f"ot{kk}", bufs=1)
                        nc.vector.tensor_tensor(ot_k[:], at[t][:],
                                                ut[t][:], Alu.mult)
                        acc["dve"] += RATE_TT[wdt]
                        nc.sync.dma_start(out=out[kk], in_=ot_k[:])
                    else:
                        o = wpool.tile([NPART, wdt * FREE], f16,
                                       tag=f"o{lev}{t}",
                                       name=f"o{lev}{t}_{kk}")
                        nc.vector.tensor_tensor(o[:], at[t][:], ut[t][:],
                                                Alu.mult)
                        acc["dve"] += RATE_TT[wdt]
                        ot.append(o)
                state[kk] = (ot, wdt)

            # software-pipelined (kernel, level) wavefront: keeps each
            # engine's in-order queue stocked with independent work
            ORDER = [(0, 0), (1, 0), (0, 1), (2, 0), (0, 2), (1, 1),
                     (3, 0), (0, 3), (1, 2), (2, 1), (0, 4), (1, 3),
                     (2, 2), (3, 1), (1, 4), (2, 3), (3, 2), (2, 4),
                     (3, 3), (3, 4)]
            for kk, lev in ORDER:
                stage(kk, lev)
    nc.compile()
    return nc


_PROGRAM = None
_PROGRAM_KEY = None


def _get_program(F, n_coef):
    global _PROGRAM, _PROGRAM_KEY
    swaps_upper = {lev: tuple(bool(v) for v in F[lev]["swap"])
                   for lev in range(1, NLEV)}
    key = (tuple(sorted(swaps_upper.items())), n_coef)
    if _PROGRAM is None or _PROGRAM_KEY != key:
        _PROGRAM = _build_program(swaps_upper, n_coef)
        _PROGRAM_KEY = key
    return _PROGRAM


def _postprocess(results, F):
    full = np.empty((K, PADBP), np.float32)
    lam = F[NLEV - 1]["lam"][0]      # (K,)
    gam = F[NLEV - 1]["gam"][0]
    for core in range(NCORES):
        o = np.asarray(results[core]["out"], dtype=np.float32)
        for kk in range(KLOC):
            k = core * KLOC + kk
            w = o[kk].reshape(PADBP)
            full[k] = w / np.float32(lam[k]) + np.float32(gam[k])
    out = full[:, :BP].reshape(K, B, OH, OW, OD).transpose(1, 0, 2, 3, 4)
    return np.ascontiguousarray(out)


def kernel(**inputs):
    x = np.asarray(inputs["x"], dtype=np.float32)
    kc = np.asarray(inputs["kernel_coords"])
    ws = [np.asarray(inputs[f"w{i}"]) for i in range(5)]

    in_maps, F = _prep_inputs(x, kc, ws)
    n_coef = in_maps[0]["coef"].shape[1]
    prog = _get_program(F, n_coef)

    from concourse.bass_utils import run_bass_kernel_spmd
    res = run_bass_kernel_spmd(prog, in_maps, list(range(NCORES)))
    return _postprocess(res.results, F)
